# revision 71
# baseline (speedup 1.0000x reference)
"""DFlashAttention Trainium2 kernel: 8-way tensor-parallel over heads.

Per core c (4 heads): Q-proj (bf16, hst-stationary so Q lands in natural
[r, d] layout and the RMS stats come straight off PSUM), fused
double-RMSNorm, RoPE (batched per row-half so batches 0-1 attention
unblocks early), full attention of 64 draft queries over 4160 keys
(bf16 QK/PV with the score factorization
score = sum_e k[t,e]*(q_rope[e]*cos[t,e] + qtil[e]*ssw[t,e]),
qtil = -rot(q_rope), ssw = half-swapped sin), softmax w/o max-subtraction
(scores are O(1)), denominator via a ones-column fused into PV, and the
row-sharded Wo projection (bf16). Host sums the 8 partial outputs.

Engine budget per core (cost-model): PE ~115us (scores/PV/V-transposes/
projections), ACT ~91us (exp + copies), DVE ~97us (cos-side K muls +
copies), Pool ~77us (sin-side K muls offloaded to GPSIMD + 2nd DMA ring),
SP ~83us (kt stream + weights). All large inputs are staged as bf16 on
the host: no cast-DMAs on device, half the HBM/upload traffic vs f32.
"""

from concurrent.futures import ThreadPoolExecutor

import numpy as np
import ml_dtypes

import concourse.bacc as bacc
import concourse.tile as tile
import concourse.mybir as mybir
from concourse.bass_utils import run_bass_kernel_spmd

bf16 = ml_dtypes.bfloat16
F32 = mybir.dt.float32
BF = mybir.dt.bfloat16
AF = mybir.ActivationFunctionType
AXX = mybir.AxisListType.X
MUL = mybir.AluOpType.mult
ADD = mybir.AluOpType.add

B, QL, CTX, HID, H, D = 4, 64, 4096, 4096, 32, 128
T = CTX + QL            # 4160
R = B * QL              # 256
HL = 4                  # heads per core
NC = 8                  # cores
EPS = 1e-6
SCALE = 1.0 / float(np.sqrt(D))
NT_FULL = T // 128      # 32 full t-tiles
TW_LAST = T - NT_FULL * 128  # 64


def build_program():
    nc = bacc.Bacc("TRN2", target_bir_lowering=False, debug=False)

    kt = nc.dram_tensor("kt", [B * HL, 128, T], BF, kind="ExternalInput")
    hst = nc.dram_tensor("hst", [HID, R], BF, kind="ExternalInput")
    wq = nc.dram_tensor("wq", [HID, HL * D], BF, kind="ExternalInput")
    wo = nc.dram_tensor("wo", [HL * D, HID], BF, kind="ExternalInput")
    cost = nc.dram_tensor("cost", [128, T], BF, kind="ExternalInput")
    sinsw = nc.dram_tensor("sinsw", [128, T], BF, kind="ExternalInput")
    cqn = nc.dram_tensor("cqn", [128, 128], F32, kind="ExternalInput")
    sqn = nc.dram_tensor("sqn", [128, 128], F32, kind="ExternalInput")
    eyeb = nc.dram_tensor("eyeb", [128, 128], BF, kind="ExternalInput")
    outp = nc.dram_tensor("outp", [R, HID], BF, kind="ExternalOutput")

    with tile.TileContext(nc) as tc:
        with (
            tc.tile_pool(name="singles", bufs=1) as sg,
            tc.tile_pool(name="ktp", bufs=3) as ktp,
            tc.tile_pool(name="kcp", bufs=3) as kcp,
            tc.tile_pool(name="ksp", bufs=3) as ksp,
        ):
            # ---- long-lived small tensors (gpsimd ring: keep SP ring free
            # for the phase-B weights and the kt stream) ----
            # cost/sinsw DMAs are issued inside phase B, after the Pool-ring
            # weight chunks (they are needed later than the weights)
            cost_bf = sg.tile([128, T], BF)
            sinsw_bf = sg.tile([128, T], BF)
            cqn_sb = sg.tile([128, 128], F32)
            nc.gpsimd.dma_start(out=cqn_sb, in_=cqn.ap())
            sqn_sb = sg.tile([128, 128], F32)
            nc.gpsimd.dma_start(out=sqn_sb, in_=sqn.ap())
            eyeb_sb = sg.tile([128, 128], BF)
            nc.gpsimd.dma_start(out=eyeb_sb, in_=eyeb.ap())
            # broadcast cos/sin rope tiles for batched rope math (built on
            # ACT during the idle DMA head); sqn8 carries the rotate_half
            # sign: rows 0:64 negated
            cqn8 = sg.tile([128, 2 * HL, 128], F32, tag="cqn8")
            sqn8 = sg.tile([128, 2 * HL, 128], F32, tag="sqn8")
            for i in range(2 * HL):
                nc.scalar.copy(out=cqn8[:, i, :], in_=cqn_sb)
                nc.scalar.activation(
                    out=sqn8[:, i, 0:64], in_=sqn_sb[:, 0:64],
                    func=AF.Copy, scale=-1.0)
                nc.scalar.copy(out=sqn8[:, i, 64:128], in_=sqn_sb[:, 64:128])
            oT_all = sg.tile([128, HL, 2, 128], BF, tag="oT")
            eps2_sb = sg.tile([128, 1], F32, tag="eps2")
            nc.vector.memset(eps2_sb, EPS * EPS)
            qc_sb = [sg.tile([128, R], BF, tag=f"qc{h}", name=f"qc{h}") for h in range(HL)]
            qs_sb = [sg.tile([128, R], BF, tag=f"qs{h}", name=f"qs{h}") for h in range(HL)]
            # big weight slot: Wq first, then Wo reuses it
            wq_sb = sg.tile([128, 32, HL * D], BF, tag="bigw")

            # =========== Phase B: Q path ===========
            # Q-proj with hst as stationary: psq comes out in natural
            # [r, d] layout, so the norm reduce needs no transpose.
            with (
                tc.tile_pool(name="bsb", bufs=2) as bsb,
                tc.tile_pool(name="hstp", bufs=1) as hstp,
                tc.tile_pool(name="psB", bufs=2, space="PSUM") as psB,
                tc.tile_pool(name="psBt", bufs=2, space="PSUM") as psBt,
            ):
                # chunked weight loads split across SP and Pool rings so the
                # k-loop starts on chunk 0 and the SP ring frees up early
                hst_sb = hstp.tile([128, 32, R], BF)

                def _hst_chunk(hc, nk, eng):  # nk k-blocks per chunk
                    nc_sync = eng
                    nc_sync.dma_start(
                        out=hst_sb[:, hc * nk:(hc + 1) * nk, :],
                        in_=hst.ap()[hc * nk * 128:(hc + 1) * nk * 128, :]
                        .rearrange("(k p) r -> p k r", p=128),
                    )

                def _wq_chunk(wc, nk, eng):
                    eng.dma_start(
                        out=wq_sb[:, wc * nk:(wc + 1) * nk, :],
                        in_=wq.ap()[wc * nk * 128:(wc + 1) * nk * 128, :]
                        .rearrange("(k p) m -> p k m", p=128),
                    )

                # fine-grained head: k-blocks 0-15 arrive in 4k steps on the
                # SP ring so Qproj starts ~3us in; k 16-31 via the Pool ring
                _hst_chunk(0, 8, nc.sync)      # hst k0-7
                _wq_chunk(0, 4, nc.sync)       # wq  k0-3
                _hst_chunk(1, 16, nc.gpsimd)   # hst k16-31
                _wq_chunk(1, 4, nc.sync)       # wq  k4-7
                _wq_chunk(2, 8, nc.gpsimd)     # wq  k16-23
                _hst_chunk(1, 8, nc.sync)      # hst k8-15
                _wq_chunk(2, 4, nc.sync)       # wq  k8-11
                _wq_chunk(3, 4, nc.sync)       # wq  k12-15
                _wq_chunk(3, 8, nc.gpsimd)     # wq  k24-31
                nc.gpsimd.dma_start(out=cost_bf, in_=cost.ap())
                nc.gpsimd.dma_start(out=sinsw_bf, in_=sinsw.ap())

                psqs = []
                for rh in range(2):
                    psq = psB.tile([128, HL, D], F32, tag=f"psq{rh}",
                                   name=f"psq{rh}")
                    for k in range(32):
                        nc.tensor.matmul(
                            psq,
                            hst_sb[:, k, rh * 128:(rh + 1) * 128],
                            wq_sb[:, k, :],
                            start=(k == 0),
                            stop=(k == 31),
                        )
                    psqs.append(psq)
                # The whole q-chain is split by row-half (rh) so batches 0-1
                # attention unblocks as soon as rh0 finishes.
                ssq = bsb.tile([128, 2 * HL], F32, tag="ssq")
                fcol = bsb.tile([128, 2 * HL], F32, tag="fcol")
                qn_all = bsb.tile([128, 2 * HL, D], F32, tag="qn")
                qcn = bsb.tile([128, 2 * HL, D], BF, tag="qcn")
                qsn = bsb.tile([128, 2 * HL, D], BF, tag="qsn")
                a_t = bsb.tile([128, 2 * HL, D], F32, tag="ropeA")
                b_t = bsb.tile([128, 2 * HL, D], F32, tag="ropeB")
                for rh in range(2):
                    sl = slice(rh * HL, (rh + 1) * HL)
                    # RMS sums straight off PSUM via accum_out (free-axis
                    # sum; single-PSUM-operand rule forces this onto ACT)
                    for h in range(HL):
                        i = rh * HL + h
                        sq_scr = bsb.tile([128, D], F32, tag="sqscr")
                        nc.scalar.activation(
                            out=sq_scr, in_=psqs[rh][:, h, :], func=AF.Square,
                            accum_out=ssq[:, i:i + 1],
                        )
                    # double-RMSNorm(w=1) == x*rsqrt((1+eps)*m + eps^2)
                    nc.scalar.activation(
                        out=fcol[:, sl], in_=ssq[:, sl], func=AF.Sqrt,
                        scale=(1.0 + EPS) / D, bias=eps2_sb[:, 0:1],
                    )
                    nc.vector.reciprocal(fcol[:, sl], fcol[:, sl])
                    # normalize on ACT (per-partition scale AP, reads PSUM)
                    for h in range(HL):
                        i = rh * HL + h
                        nc.scalar.activation(
                            out=qn_all[:, i, :], in_=psqs[rh][:, h, :],
                            func=AF.Copy, scale=fcol[:, i:i + 1],
                        )
                    # rope, batched per rh (sign of rotate_half in sqn8)
                    nc.vector.tensor_mul(a_t[:, sl], qn_all[:, sl], cqn8[:, sl])
                    nc.vector.tensor_mul(
                        b_t[:, sl, 0:64], qn_all[:, sl, 64:128],
                        sqn8[:, sl, 0:64])
                    nc.vector.tensor_mul(
                        b_t[:, sl, 64:128], qn_all[:, sl, 0:64],
                        sqn8[:, sl, 64:128])
                    nc.vector.tensor_add(qcn[:, sl], a_t[:, sl], b_t[:, sl])
                    # qtil = -rot(q_rope)
                    nc.vector.tensor_copy(
                        qsn[:, sl, 0:64], qcn[:, sl, 64:128])
                    nc.vector.tensor_scalar_mul(
                        qsn[:, sl, 64:128], qcn[:, sl, 0:64], -1.0)
                    for h in range(HL):
                        i = rh * HL + h
                        pqc = psBt.tile([128, 128], BF, tag="pqc")
                        pqs = psBt.tile([128, 128], BF, tag="pqs")
                        nc.tensor.transpose(pqc, qcn[:, i, :], eyeb_sb)
                        nc.tensor.transpose(pqs, qsn[:, i, :], eyeb_sb)
                        nc.scalar.copy(
                            out=qc_sb[h][:, rh * 128:(rh + 1) * 128], in_=pqc)
                        nc.vector.tensor_copy(
                            out=qs_sb[h][:, rh * 128:(rh + 1) * 128], in_=pqs)

            # =========== Phase C: attention over K ===========
            with (
                tc.tile_pool(name="expp", bufs=4) as expp,
                tc.tile_pool(name="vp", bufs=4) as vp,
                tc.tile_pool(name="osm", bufs=2) as osm,
                tc.tile_pool(name="psSC", bufs=2, space="PSUM") as psSC,
                tc.tile_pool(name="psV", bufs=3, space="PSUM") as psV,
                tc.tile_pool(name="psO", bufs=2, space="PSUM") as psO,
                tc.tile_pool(name="psOT", bufs=1, space="PSUM") as psOT,
            ):
                # wo loads as 4 chunks interleaved into the kt stream (SP
                # ring) so neither the Pool muls nor the kt cadence block
                wo_sb = sg.tile([128, HL, HID], BF, tag="bigw")
                wo_issue = {6: 0, 9: 1, 12: 2, 14: 3}
                for i in range(B * HL):
                    b, h = divmod(i, HL)
                    if i in wo_issue:
                        c = wo_issue[i]
                        nc.sync.dma_start(
                            out=wo_sb[:, c, :],
                            in_=wo.ap()[c * 128:(c + 1) * 128, :],
                        )
                    kt_bf = ktp.tile([128, T], BF, tag="kt")
                    nc.sync.dma_start(out=kt_bf, in_=kt.ap()[i])
                    kc = kcp.tile([128, T], BF, tag="kc")
                    # DVE is congested by the q-chain early on; shift a
                    # couple of cos-side muls to Pool there
                    if i in (2, 6, 10):
                        nc.gpsimd.tensor_mul(kc, kt_bf, cost_bf)
                    else:
                        nc.vector.tensor_mul(kc, kt_bf, cost_bf)
                    ks = ksp.tile([128, T], BF, tag="ks")
                    # Pool engine is otherwise idle; offload the sin-side mul
                    nc.gpsimd.tensor_mul(ks, kt_bf, sinsw_bf)

                    po = psO.tile([64, 129], F32, tag="po")
                    qcb = qc_sb[h][:, b * QL:(b + 1) * QL]
                    qsb_ = qs_sb[h][:, b * QL:(b + 1) * QL]

                    ntiles = NT_FULL + 1
                    for g in range((ntiles + 7) // 8):  # groups of 8 t-tiles
                        j0, j1 = g * 8, min(ntiles, g * 8 + 8)
                        gw = j1 - j0
                        psc = psSC.tile([128, 8, QL], F32, tag="psc")
                        for jj in range(gw):
                            j = j0 + jj
                            t0 = j * 128
                            tw = 128 if j < NT_FULL else TW_LAST
                            nc.tensor.matmul(
                                psc[:tw, jj, :], kc[:, t0:t0 + tw], qcb,
                                start=True, stop=False,
                            )
                            nc.tensor.matmul(
                                psc[:tw, jj, :], ks[:, t0:t0 + tw], qsb_,
                                start=False, stop=True,
                            )
                        exp_sb = expp.tile([128, 8, QL], BF, tag="exp")
                        if j1 == ntiles:  # last group contains the 64-wide tile
                            if gw > 1:
                                nc.scalar.activation(
                                    out=exp_sb[:, 0:gw - 1, :], in_=psc[:, 0:gw - 1, :],
                                    func=AF.Exp, scale=SCALE,
                                )
                            nc.scalar.activation(
                                out=exp_sb[:TW_LAST, gw - 1, :], in_=psc[:TW_LAST, gw - 1, :],
                                func=AF.Exp, scale=SCALE,
                            )
                        else:
                            nc.scalar.activation(
                                out=exp_sb[:, 0:gw, :], in_=psc[:, 0:gw, :],
                                func=AF.Exp, scale=SCALE,
                            )
                        # V tiles: transpose kt back to [t, d], batches of 4
                        for vg in range((gw + 3) // 4):
                            vj0, vj1 = j0 + vg * 4, min(j1, j0 + vg * 4 + 4)
                            pv = psV.tile([128, 4, 128], BF, tag="pv")
                            for jj in range(vj1 - vj0):
                                j = vj0 + jj
                                t0 = j * 128
                                tw = 128 if j < NT_FULL else TW_LAST
                                nc.tensor.transpose(
                                    pv[:tw, jj, :], kt_bf[:, t0:t0 + tw], eyeb_sb
                                )
                            v_sb = vp.tile([128, 4, 130], BF, tag="v")
                            nvw = vj1 - vj0
                            # alternate copy engine to balance ACT vs DVE
                            # (DVE for the first pairs: ACT is congested by
                            # the q-chain tail at the phase transition)
                            if (i + vg) % 2 == 0:
                                nc.scalar.copy(
                                    out=v_sb[:, 0:nvw, 0:128], in_=pv[:, 0:nvw, :]
                                )
                            else:
                                nc.vector.tensor_copy(
                                    out=v_sb[:, 0:nvw, 0:128], in_=pv[:, 0:nvw, :]
                                )
                            nc.vector.memset(v_sb[:, 0:nvw, 128:129], 1.0)
                            for jj in range(nvw):
                                j = vj0 + jj
                                tw = 128 if j < NT_FULL else TW_LAST
                                nc.tensor.matmul(
                                    po,
                                    exp_sb[:tw, j - j0, :],
                                    v_sb[:tw, jj, 0:129],
                                    start=(j == 0),
                                    stop=(j == ntiles - 1),
                                )
                    # normalize + transpose out
                    rec = osm.tile([64, 1], F32, tag="rec")
                    nc.vector.reciprocal(rec, po[:, 128:129])
                    onrm = osm.tile([64, 128], BF, tag="onrm")
                    nc.vector.tensor_scalar_mul(onrm, po[:, 0:128], rec)
                    poT = psOT.tile([128, 64], BF, tag="poT")
                    nc.tensor.transpose(poT, onrm, eyeb_sb[0:64, 0:64])
                    bp, bo = divmod(b, 2)
                    nc.vector.tensor_copy(
                        out=oT_all[:, h, bp, bo * 64:(bo + 1) * 64], in_=poT)

            # =========== Phase E: output projection ===========
            with (
                tc.tile_pool(name="obp", bufs=2) as obp,
                tc.tile_pool(name="psW", bufs=4, space="PSUM") as psW,
            ):
                for bp in range(2):  # batch pairs -> 128 output rows each
                    ob = obp.tile([128, HID], BF, tag="ob")
                    for oc in range(8):
                        pw = psW.tile([128, 512], F32, tag="pw")
                        for hd in range(HL):
                            nc.tensor.matmul(
                                pw,
                                oT_all[:, hd, bp, :],
                                wo_sb[:, hd, oc * 512:(oc + 1) * 512],
                                start=(hd == 0),
                                stop=(hd == HL - 1),
                            )
                        nc.vector.tensor_copy(
                            out=ob[:, oc * 512:(oc + 1) * 512], in_=pw,
                        )
                        if oc % 4 == 3:  # stream the output out in halves
                            nc.sync.dma_start(
                                out=outp.ap()[
                                    bp * 128:(bp + 1) * 128,
                                    (oc - 3) * 512:(oc + 1) * 512],
                                in_=ob[:, (oc - 3) * 512:(oc + 1) * 512],
                            )
    nc.compile()
    return nc


_PROGRAM = None


def get_program():
    global _PROGRAM
    if _PROGRAM is None:
        _PROGRAM = build_program()
    return _PROGRAM


def stage_inputs(hidden_states, target_hidden, cos, sin, Wqkv, Wo, q_norm_w):
    """Host-side shard + layout staging (all-bf16). Returns list of 8 in_maps."""
    assert np.allclose(np.asarray(q_norm_w), 1.0), "kernel assumes q_norm_w == ones"
    hs16 = np.asarray(hidden_states, np.float32).astype(bf16)
    th16 = np.asarray(target_hidden, np.float32).astype(bf16)
    cos = np.asarray(cos, np.float32)
    sin = np.asarray(sin, np.float32)
    Wqkv32 = np.asarray(Wqkv, np.float32)
    Wo16 = np.asarray(Wo, np.float32).astype(bf16)

    hst = np.ascontiguousarray(hs16.reshape(R, HID).T)           # [HID, R] bf16
    cost = np.ascontiguousarray(cos.T).astype(bf16)              # [128, T]
    sinsw = np.ascontiguousarray(np.roll(sin.T, -64, axis=0)).astype(bf16)
    cqn = np.ascontiguousarray(np.tile(cos[CTX:], (2, 1)))       # [128, 128] f32
    sqn = np.ascontiguousarray(np.tile(sin[CTX:], (2, 1)))
    eyeb = np.eye(128, dtype=bf16)

    def build_core(c):
        cs = c * HL * D
        kt = np.empty((B * HL, D, T), bf16)
        for b in range(B):
            for h in range(HL):
                col = cs + h * D
                kt[b * HL + h, :, :CTX] = th16[b, :, col:col + D].T
                kt[b * HL + h, :, CTX:] = hs16[b, :, col:col + D].T
        return {
            "kt": kt,
            "hst": hst,
            "wq": Wqkv32[:, cs:cs + HL * D].astype(bf16),
            "wo": np.ascontiguousarray(Wo16[cs:cs + HL * D, :]),
            "cost": cost,
            "sinsw": sinsw,
            "cqn": cqn,
            "sqn": sqn,
            "eyeb": eyeb,
        }

    with ThreadPoolExecutor(NC) as ex:
        in_maps = list(ex.map(build_core, range(NC)))
    return in_maps


def kernel(hidden_states, target_hidden, cos, sin, Wqkv, Wo, q_norm_w):
    nc = get_program()
    in_maps = stage_inputs(hidden_states, target_hidden, cos, sin, Wqkv, Wo, q_norm_w)
    res = run_bass_kernel_spmd(nc, in_maps, core_ids=list(range(NC)))
    out = np.zeros((R, HID), np.float32)
    for r in res.results:
        out += r["outp"].astype(np.float32)
    return out.reshape(B, QL, HID)


# revision 75
# speedup vs baseline: 3.8418x; 3.8418x over previous
"""DFlashAttention Trainium2 kernel: 8-way tensor-parallel over heads.

Per core c (4 heads): Q-proj (bf16, hst-stationary so Q lands in natural
[r, d] layout and the RMS stats come straight off PSUM), fused
double-RMSNorm, RoPE (batched per row-half so batches 0-1 attention
unblocks early), full attention of 64 draft queries over 4160 keys
(bf16 QK/PV with the score factorization
score = sum_e k[t,e]*(q_rope[e]*cos[t,e] + qtil[e]*ssw[t,e]),
qtil = -rot(q_rope), ssw = half-swapped sin), softmax w/o max-subtraction
(scores are O(1)), denominator via a ones-column fused into PV, and the
row-sharded Wo projection (bf16). Host sums the 8 partial outputs.

Engine budget per core (cost-model): PE ~115us (scores/PV/V-transposes/
projections), ACT ~91us (exp + copies), DVE ~97us (cos-side K muls +
copies), Pool ~77us (sin-side K muls offloaded to GPSIMD + 2nd DMA ring),
SP ~83us (kt stream + weights). All large inputs are staged as bf16 on
the host: no cast-DMAs on device, half the HBM/upload traffic vs f32.
"""

from concurrent.futures import ThreadPoolExecutor

import numpy as np
import ml_dtypes

import concourse.bacc as bacc
import concourse.tile as tile
import concourse.mybir as mybir
from concourse.bass_utils import run_bass_kernel_spmd

bf16 = ml_dtypes.bfloat16
F32 = mybir.dt.float32
BF = mybir.dt.bfloat16
AF = mybir.ActivationFunctionType
AXX = mybir.AxisListType.X
MUL = mybir.AluOpType.mult
ADD = mybir.AluOpType.add

B, QL, CTX, HID, H, D = 4, 64, 4096, 4096, 32, 128
T = CTX + QL            # 4160
R = B * QL              # 256
HL = 4                  # heads per core
NC = 8                  # cores
EPS = 1e-6
SCALE = 1.0 / float(np.sqrt(D))
NT_FULL = T // 128      # 32 full t-tiles
TW_LAST = T - NT_FULL * 128  # 64


def build_program():
    nc = bacc.Bacc("TRN2", target_bir_lowering=False, debug=False)

    kt = nc.dram_tensor("kt", [B * HL, 128, T], BF, kind="ExternalInput")
    hst = nc.dram_tensor("hst", [HID, R], BF, kind="ExternalInput")
    wq = nc.dram_tensor("wq", [HID, HL * D], BF, kind="ExternalInput")
    wo = nc.dram_tensor("wo", [HL * D, HID], BF, kind="ExternalInput")
    cost = nc.dram_tensor("cost", [128, T], BF, kind="ExternalInput")
    sinsw = nc.dram_tensor("sinsw", [128, T], BF, kind="ExternalInput")
    cqn = nc.dram_tensor("cqn", [128, 128], F32, kind="ExternalInput")
    sqn = nc.dram_tensor("sqn", [128, 128], F32, kind="ExternalInput")
    eyeb = nc.dram_tensor("eyeb", [128, 128], BF, kind="ExternalInput")
    outp = nc.dram_tensor("outp", [R, HID], BF, kind="ExternalOutput")

    with tile.TileContext(nc) as tc:
        with (
            tc.tile_pool(name="singles", bufs=1) as sg,
            tc.tile_pool(name="ktp", bufs=3) as ktp,
            tc.tile_pool(name="kcp", bufs=3) as kcp,
            tc.tile_pool(name="ksp", bufs=3) as ksp,
        ):
            # ---- long-lived small tensors (gpsimd ring: keep SP ring free
            # for the phase-B weights and the kt stream) ----
            # cost/sinsw DMAs are issued inside phase B, after the Pool-ring
            # weight chunks (they are needed later than the weights)
            cost_bf = sg.tile([128, T], BF)
            sinsw_bf = sg.tile([128, T], BF)
            cqn_sb = sg.tile([128, 128], F32)
            nc.gpsimd.dma_start(out=cqn_sb, in_=cqn.ap())
            sqn_sb = sg.tile([128, 128], F32)
            nc.gpsimd.dma_start(out=sqn_sb, in_=sqn.ap())
            eyeb_sb = sg.tile([128, 128], BF)
            nc.gpsimd.dma_start(out=eyeb_sb, in_=eyeb.ap())
            # broadcast cos/sin rope tiles for batched rope math (built on
            # ACT during the idle DMA head); sqn8 carries the rotate_half
            # sign: rows 0:64 negated
            cqn8 = sg.tile([128, 2 * HL, 128], F32, tag="cqn8")
            sqn8 = sg.tile([128, 2 * HL, 128], F32, tag="sqn8")
            for i in range(2 * HL):
                nc.scalar.copy(out=cqn8[:, i, :], in_=cqn_sb)
                nc.scalar.activation(
                    out=sqn8[:, i, 0:64], in_=sqn_sb[:, 0:64],
                    func=AF.Copy, scale=-1.0)
                nc.scalar.copy(out=sqn8[:, i, 64:128], in_=sqn_sb[:, 64:128])
            oT_all = sg.tile([128, HL, 2, 128], BF, tag="oT")
            eps2_sb = sg.tile([128, 1], F32, tag="eps2")
            nc.vector.memset(eps2_sb, EPS * EPS)
            qc_sb = [sg.tile([128, R], BF, tag=f"qc{h}", name=f"qc{h}") for h in range(HL)]
            qs_sb = [sg.tile([128, R], BF, tag=f"qs{h}", name=f"qs{h}") for h in range(HL)]
            # big weight slot: Wq first, then Wo reuses it
            wq_sb = sg.tile([128, 32, HL * D], BF, tag="bigw")

            # =========== Phase B: Q path ===========
            # Q-proj with hst as stationary: psq comes out in natural
            # [r, d] layout, so the norm reduce needs no transpose.
            with (
                tc.tile_pool(name="bsb", bufs=2) as bsb,
                tc.tile_pool(name="hstp", bufs=1) as hstp,
                tc.tile_pool(name="psB", bufs=2, space="PSUM") as psB,
                tc.tile_pool(name="psBt", bufs=2, space="PSUM") as psBt,
            ):
                # chunked weight loads split across SP and Pool rings so the
                # k-loop starts on chunk 0 and the SP ring frees up early
                hst_sb = hstp.tile([128, 32, R], BF)

                def _hst_chunk(hc, nk, eng):  # nk k-blocks per chunk
                    nc_sync = eng
                    nc_sync.dma_start(
                        out=hst_sb[:, hc * nk:(hc + 1) * nk, :],
                        in_=hst.ap()[hc * nk * 128:(hc + 1) * nk * 128, :]
                        .rearrange("(k p) r -> p k r", p=128),
                    )

                def _wq_chunk(wc, nk, eng):
                    eng.dma_start(
                        out=wq_sb[:, wc * nk:(wc + 1) * nk, :],
                        in_=wq.ap()[wc * nk * 128:(wc + 1) * nk * 128, :]
                        .rearrange("(k p) m -> p k m", p=128),
                    )

                # fine-grained head: k-blocks 0-15 arrive in 4k steps on the
                # SP ring so Qproj starts ~3us in; k 16-31 via the Pool ring
                _hst_chunk(0, 8, nc.sync)      # hst k0-7
                _wq_chunk(0, 4, nc.sync)       # wq  k0-3
                _hst_chunk(1, 16, nc.gpsimd)   # hst k16-31
                _wq_chunk(1, 4, nc.sync)       # wq  k4-7
                _wq_chunk(2, 8, nc.gpsimd)     # wq  k16-23
                _hst_chunk(1, 8, nc.sync)      # hst k8-15
                _wq_chunk(2, 4, nc.sync)       # wq  k8-11
                _wq_chunk(3, 4, nc.sync)       # wq  k12-15
                _wq_chunk(3, 8, nc.gpsimd)     # wq  k24-31
                nc.gpsimd.dma_start(out=cost_bf, in_=cost.ap())
                nc.gpsimd.dma_start(out=sinsw_bf, in_=sinsw.ap())

                psqs = []
                for rh in range(2):
                    psq = psB.tile([128, HL, D], F32, tag=f"psq{rh}",
                                   name=f"psq{rh}")
                    for k in range(32):
                        nc.tensor.matmul(
                            psq,
                            hst_sb[:, k, rh * 128:(rh + 1) * 128],
                            wq_sb[:, k, :],
                            start=(k == 0),
                            stop=(k == 31),
                        )
                    psqs.append(psq)
                # The whole q-chain is split by row-half (rh) so batches 0-1
                # attention unblocks as soon as rh0 finishes.
                ssq = bsb.tile([128, 2 * HL], F32, tag="ssq")
                fcol = bsb.tile([128, 2 * HL], F32, tag="fcol")
                qn_all = bsb.tile([128, 2 * HL, D], F32, tag="qn")
                qcn = bsb.tile([128, 2 * HL, D], BF, tag="qcn")
                qsn = bsb.tile([128, 2 * HL, D], BF, tag="qsn")
                a_t = bsb.tile([128, 2 * HL, D], F32, tag="ropeA")
                b_t = bsb.tile([128, 2 * HL, D], F32, tag="ropeB")
                for rh in range(2):
                    sl = slice(rh * HL, (rh + 1) * HL)
                    # RMS sums straight off PSUM via accum_out (free-axis
                    # sum; single-PSUM-operand rule forces this onto ACT)
                    for h in range(HL):
                        i = rh * HL + h
                        sq_scr = bsb.tile([128, D], F32, tag="sqscr")
                        nc.scalar.activation(
                            out=sq_scr, in_=psqs[rh][:, h, :], func=AF.Square,
                            accum_out=ssq[:, i:i + 1],
                        )
                    # double-RMSNorm(w=1) == x*rsqrt((1+eps)*m + eps^2)
                    nc.scalar.activation(
                        out=fcol[:, sl], in_=ssq[:, sl], func=AF.Sqrt,
                        scale=(1.0 + EPS) / D, bias=eps2_sb[:, 0:1],
                    )
                    nc.vector.reciprocal(fcol[:, sl], fcol[:, sl])
                    # normalize on ACT (per-partition scale AP, reads PSUM)
                    for h in range(HL):
                        i = rh * HL + h
                        nc.scalar.activation(
                            out=qn_all[:, i, :], in_=psqs[rh][:, h, :],
                            func=AF.Copy, scale=fcol[:, i:i + 1],
                        )
                    # rope, batched per rh (sign of rotate_half in sqn8)
                    nc.vector.tensor_mul(a_t[:, sl], qn_all[:, sl], cqn8[:, sl])
                    nc.vector.tensor_mul(
                        b_t[:, sl, 0:64], qn_all[:, sl, 64:128],
                        sqn8[:, sl, 0:64])
                    nc.vector.tensor_mul(
                        b_t[:, sl, 64:128], qn_all[:, sl, 0:64],
                        sqn8[:, sl, 64:128])
                    nc.vector.tensor_add(qcn[:, sl], a_t[:, sl], b_t[:, sl])
                    # qtil = -rot(q_rope)
                    nc.vector.tensor_copy(
                        qsn[:, sl, 0:64], qcn[:, sl, 64:128])
                    nc.vector.tensor_scalar_mul(
                        qsn[:, sl, 64:128], qcn[:, sl, 0:64], -1.0)
                    for h in range(HL):
                        i = rh * HL + h
                        pqc = psBt.tile([128, 128], BF, tag="pqc")
                        pqs = psBt.tile([128, 128], BF, tag="pqs")
                        nc.tensor.transpose(pqc, qcn[:, i, :], eyeb_sb)
                        nc.tensor.transpose(pqs, qsn[:, i, :], eyeb_sb)
                        nc.scalar.copy(
                            out=qc_sb[h][:, rh * 128:(rh + 1) * 128], in_=pqc)
                        nc.vector.tensor_copy(
                            out=qs_sb[h][:, rh * 128:(rh + 1) * 128], in_=pqs)

            # =========== Phase C: attention over K ===========
            with (
                tc.tile_pool(name="expp", bufs=4) as expp,
                tc.tile_pool(name="vp", bufs=4) as vp,
                tc.tile_pool(name="osm", bufs=2) as osm,
                tc.tile_pool(name="psSC", bufs=2, space="PSUM") as psSC,
                tc.tile_pool(name="psV", bufs=3, space="PSUM") as psV,
                tc.tile_pool(name="psO", bufs=2, space="PSUM") as psO,
                tc.tile_pool(name="psOT", bufs=1, space="PSUM") as psOT,
            ):
                # wo loads as 4 chunks interleaved into the kt stream (SP
                # ring) so neither the Pool muls nor the kt cadence block
                wo_sb = sg.tile([128, HL, HID], BF, tag="bigw")
                wo_issue = {6: 0, 9: 1, 12: 2, 14: 3}
                for i in range(B * HL):
                    b, h = divmod(i, HL)
                    if i in wo_issue:
                        c = wo_issue[i]
                        nc.sync.dma_start(
                            out=wo_sb[:, c, :],
                            in_=wo.ap()[c * 128:(c + 1) * 128, :],
                        )
                    kt_bf = ktp.tile([128, T], BF, tag="kt")
                    nc.sync.dma_start(out=kt_bf, in_=kt.ap()[i])
                    kc = kcp.tile([128, T], BF, tag="kc")
                    # DVE is congested by the q-chain early on; shift a
                    # couple of cos-side muls to Pool there
                    if i in (2, 6, 10):
                        nc.gpsimd.tensor_mul(kc, kt_bf, cost_bf)
                    else:
                        nc.vector.tensor_mul(kc, kt_bf, cost_bf)
                    ks = ksp.tile([128, T], BF, tag="ks")
                    # Pool engine is otherwise idle; offload the sin-side mul
                    nc.gpsimd.tensor_mul(ks, kt_bf, sinsw_bf)

                    po = psO.tile([64, 129], F32, tag="po")
                    qcb = qc_sb[h][:, b * QL:(b + 1) * QL]
                    qsb_ = qs_sb[h][:, b * QL:(b + 1) * QL]

                    ntiles = NT_FULL + 1
                    for g in range((ntiles + 7) // 8):  # groups of 8 t-tiles
                        j0, j1 = g * 8, min(ntiles, g * 8 + 8)
                        gw = j1 - j0
                        psc = psSC.tile([128, 8, QL], F32, tag="psc")
                        for jj in range(gw):
                            j = j0 + jj
                            t0 = j * 128
                            tw = 128 if j < NT_FULL else TW_LAST
                            nc.tensor.matmul(
                                psc[:tw, jj, :], kc[:, t0:t0 + tw], qcb,
                                start=True, stop=False,
                            )
                            nc.tensor.matmul(
                                psc[:tw, jj, :], ks[:, t0:t0 + tw], qsb_,
                                start=False, stop=True,
                            )
                        exp_sb = expp.tile([128, 8, QL], BF, tag="exp")
                        if j1 == ntiles:  # last group contains the 64-wide tile
                            if gw > 1:
                                nc.scalar.activation(
                                    out=exp_sb[:, 0:gw - 1, :], in_=psc[:, 0:gw - 1, :],
                                    func=AF.Exp, scale=SCALE,
                                )
                            nc.scalar.activation(
                                out=exp_sb[:TW_LAST, gw - 1, :], in_=psc[:TW_LAST, gw - 1, :],
                                func=AF.Exp, scale=SCALE,
                            )
                        else:
                            nc.scalar.activation(
                                out=exp_sb[:, 0:gw, :], in_=psc[:, 0:gw, :],
                                func=AF.Exp, scale=SCALE,
                            )
                        # V tiles: transpose kt back to [t, d], batches of 4
                        for vg in range((gw + 3) // 4):
                            vj0, vj1 = j0 + vg * 4, min(j1, j0 + vg * 4 + 4)
                            pv = psV.tile([128, 4, 128], BF, tag="pv")
                            for jj in range(vj1 - vj0):
                                j = vj0 + jj
                                t0 = j * 128
                                tw = 128 if j < NT_FULL else TW_LAST
                                nc.tensor.transpose(
                                    pv[:tw, jj, :], kt_bf[:, t0:t0 + tw], eyeb_sb
                                )
                            v_sb = vp.tile([128, 4, 130], BF, tag="v")
                            nvw = vj1 - vj0
                            # alternate copy engine to balance ACT vs DVE
                            # (DVE for the first pairs: ACT is congested by
                            # the q-chain tail at the phase transition)
                            if (i + vg) % 2 == 0:
                                nc.scalar.copy(
                                    out=v_sb[:, 0:nvw, 0:128], in_=pv[:, 0:nvw, :]
                                )
                            else:
                                nc.vector.tensor_copy(
                                    out=v_sb[:, 0:nvw, 0:128], in_=pv[:, 0:nvw, :]
                                )
                            nc.vector.memset(v_sb[:, 0:nvw, 128:129], 1.0)
                            for jj in range(nvw):
                                j = vj0 + jj
                                tw = 128 if j < NT_FULL else TW_LAST
                                nc.tensor.matmul(
                                    po,
                                    exp_sb[:tw, j - j0, :],
                                    v_sb[:tw, jj, 0:129],
                                    start=(j == 0),
                                    stop=(j == ntiles - 1),
                                )
                    # normalize + transpose out
                    rec = osm.tile([64, 1], F32, tag="rec")
                    nc.vector.reciprocal(rec, po[:, 128:129])
                    onrm = osm.tile([64, 128], BF, tag="onrm")
                    nc.vector.tensor_scalar_mul(onrm, po[:, 0:128], rec)
                    poT = psOT.tile([128, 64], BF, tag="poT")
                    nc.tensor.transpose(poT, onrm, eyeb_sb[0:64, 0:64])
                    bp, bo = divmod(b, 2)
                    nc.vector.tensor_copy(
                        out=oT_all[:, h, bp, bo * 64:(bo + 1) * 64], in_=poT)

            # =========== Phase E: output projection ===========
            with (
                tc.tile_pool(name="obp", bufs=2) as obp,
                tc.tile_pool(name="psW", bufs=4, space="PSUM") as psW,
            ):
                for bp in range(2):  # batch pairs -> 128 output rows each
                    ob = obp.tile([128, HID], BF, tag="ob")
                    for oc in range(8):
                        pw = psW.tile([128, 512], F32, tag="pw")
                        for hd in range(HL):
                            nc.tensor.matmul(
                                pw,
                                oT_all[:, hd, bp, :],
                                wo_sb[:, hd, oc * 512:(oc + 1) * 512],
                                start=(hd == 0),
                                stop=(hd == HL - 1),
                            )
                        nc.vector.tensor_copy(
                            out=ob[:, oc * 512:(oc + 1) * 512], in_=pw,
                        )
                        if oc % 4 == 3:  # stream the output out in halves
                            nc.sync.dma_start(
                                out=outp.ap()[
                                    bp * 128:(bp + 1) * 128,
                                    (oc - 3) * 512:(oc + 1) * 512],
                                in_=ob[:, (oc - 3) * 512:(oc + 1) * 512],
                            )
    nc.compile()
    return nc


_PROGRAM = None


def get_program():
    global _PROGRAM
    if _PROGRAM is None:
        _PROGRAM = build_program()
    return _PROGRAM


def stage_inputs(hidden_states, target_hidden, cos, sin, Wqkv, Wo, q_norm_w):
    """Host-side shard + layout staging (all-bf16). Returns list of 8 in_maps."""
    assert np.allclose(np.asarray(q_norm_w), 1.0), "kernel assumes q_norm_w == ones"
    hs16 = np.asarray(hidden_states, np.float32).astype(bf16)
    th16 = np.asarray(target_hidden, np.float32).astype(bf16)
    cos = np.asarray(cos, np.float32)
    sin = np.asarray(sin, np.float32)
    Wqkv32 = np.asarray(Wqkv, np.float32)
    Wo16 = np.asarray(Wo, np.float32).astype(bf16)

    hst = np.ascontiguousarray(hs16.reshape(R, HID).T)           # [HID, R] bf16
    cost = np.ascontiguousarray(cos.T).astype(bf16)              # [128, T]
    sinsw = np.ascontiguousarray(np.roll(sin.T, -64, axis=0)).astype(bf16)
    cqn = np.ascontiguousarray(np.tile(cos[CTX:], (2, 1)))       # [128, 128] f32
    sqn = np.ascontiguousarray(np.tile(sin[CTX:], (2, 1)))
    eyeb = np.eye(128, dtype=bf16)

    def build_core(c):
        cs = c * HL * D
        kt = np.empty((B * HL, D, T), bf16)
        for b in range(B):
            for h in range(HL):
                col = cs + h * D
                kt[b * HL + h, :, :CTX] = th16[b, :, col:col + D].T
                kt[b * HL + h, :, CTX:] = hs16[b, :, col:col + D].T
        return {
            "kt": kt,
            "hst": hst,
            "wq": Wqkv32[:, cs:cs + HL * D].astype(bf16),
            "wo": np.ascontiguousarray(Wo16[cs:cs + HL * D, :]),
            "cost": cost,
            "sinsw": sinsw,
            "cqn": cqn,
            "sqn": sqn,
            "eyeb": eyeb,
        }

    with ThreadPoolExecutor(NC) as ex:
        in_maps = list(ex.map(build_core, range(NC)))
    return in_maps


def kernel(hidden_states, target_hidden, cos, sin, Wqkv, Wo, q_norm_w):
    nc = get_program()
    in_maps = stage_inputs(hidden_states, target_hidden, cos, sin, Wqkv, Wo, q_norm_w)
    res = run_bass_kernel_spmd(nc, in_maps, core_ids=list(range(NC)))
    out = np.zeros((R, HID), np.float32)
    for r in res.results:
        out += r["outp"].astype(np.float32)
    return out.reshape(B, QL, HID)
